# revision 1
# baseline (speedup 1.0000x reference)
"""Multi-head self-attention (B=2, S=2048, E=1024, H=16, D=64) on 8 NeuronCores.

Sharding: core c -> (batch b = c // 4, head group g = c % 4).  Each core
computes Q/K/V projections for its 4 heads (column-parallel), attention, and
a partial output projection (row-parallel); the host sums the 4 partials per
batch.  All device activations live in "transposed space" (feature on the
partition dim) so every matmul contracts along partitions with no on-device
transposes:

  Q^T = Wq_g^T @ X^T          [256, 2048]  (e-chunk accumulated, + bq)
  K^T = Wk_g^T @ X^T          [256, 2048]
  V   = X @ Wv_g              [2048, 256]  (natural; ones column appended)
  S^T = K_h @ Q_h^T / 8       [2048, 2048] per head (computed tile-wise)
  P^T = exp(S^T)              (softmax without max-subtraction: scores ~N(0,1))
  O'^T = [V_h | 1]^T @ P^T    [65, q]  (row 64 = softmax denominators)
  O^T  = O'[0:64] / O'[64]    (DVE reciprocal + GpSimd partition broadcast)
  Y^T  = Wo_g^T @ O^T         [1024, 2048] partial, host-summed per batch

bv and bo are folded on the host (exact: softmax rows sum to 1, so
attn(V + bv) = attn(V) + bv, and the output projection is linear).
"""

from contextlib import ExitStack

import numpy as np

import concourse.bass as bass
import concourse.tile as tile
from concourse import bacc, mybir
from concourse.bass_utils import run_bass_kernel_spmd

B, S, E, H, D = 2, 2048, 1024, 16, 64
NCORES = 8
GH = 4            # heads per core
DC = GH * D       # head-dim columns per core (256)
EC = E // 128     # 8 e-chunks
KC = S // 128     # 16 k-chunks
F32 = mybir.dt.float32
MM_DT = mybir.dt.float16    # full-speed 16-bit matmul path (10-bit mantissa)
EXP_FUNC = mybir.ActivationFunctionType.Exp
SCALE = 1.0 / np.sqrt(np.float32(D))


def _mm(ap):
    return ap


def round_f32r(a):
    # Host-side conversion to the matmul dtype (RNE)
    if MM_DT == mybir.dt.float16:
        return np.ascontiguousarray(a, np.float32).astype(np.float16)
    if MM_DT == mybir.dt.bfloat16:
        import ml_dtypes
        return np.ascontiguousarray(a, np.float32).astype(ml_dtypes.bfloat16)
    if MM_DT == mybir.dt.float32r:
        u = np.ascontiguousarray(a, np.float32).view(np.uint32)
        u = ((u.astype(np.uint64) + 0x800) & 0xFFFFF000).astype(np.uint32)
        return u.view(np.float32)
    return np.ascontiguousarray(a, np.float32)


DEBUG_DUMPS = False


def _emit(nc, tc, ctx, xT, wq, wk, wv, wo, bq, bk, yT, dbg=None):
    sb_big = ctx.enter_context(tc.tile_pool(name="sb_big", bufs=1))
    sb_p = ctx.enter_context(tc.tile_pool(name="sb_p", bufs=28))
    sb_norm = ctx.enter_context(tc.tile_pool(name="sb_norm", bufs=4))
    sb_y = ctx.enter_context(tc.tile_pool(name="sb_y", bufs=2))
    ps_big = ctx.enter_context(tc.tile_pool(name="ps_big", bufs=2, space="PSUM"))
    ps_acc = ctx.enter_context(tc.tile_pool(name="ps_acc", bufs=2, space="PSUM"))

    xT_t = sb_big.tile([128, EC, S], MM_DT)
    wq_t = sb_big.tile([128, EC, DC], MM_DT)
    wk_t = sb_big.tile([128, EC, DC], MM_DT)
    wv_t = sb_big.tile([128, EC, DC], MM_DT)
    wo_t = sb_big.tile([128, 2, E], MM_DT)
    bqk_t = sb_big.tile([1, 2, DC], MM_DT)
    ones_t = sb_big.tile([1, 512], MM_DT)
    qT_t = sb_big.tile([128, 2, S], MM_DT)
    kT_t = sb_big.tile([128, 2, S], MM_DT)
    v_t = sb_big.tile([128, KC, GH, D + 1], MM_DT)
    o_t = sb_big.tile([128, 2, S], MM_DT)

    # Inputs are host-permuted to the exact SBUF layouts, so every load is a
    # dense per-partition-contiguous copy (cheap descriptors); issues are
    # spread across engine queues to parallelize DMA setup.
    nc.scalar.dma_start(out=wq_t[:, :, :],
                        in_=wq.rearrange("p (c d) -> p c d", c=EC))
    nc.scalar.dma_start(out=wk_t[:, :, :],
                        in_=wk.rearrange("p (c d) -> p c d", c=EC))
    for ec in range(EC):
        eng = nc.sync if ec % 2 == 0 else nc.gpsimd
        eng.dma_start(out=xT_t[:, ec, :], in_=xT[:, ec * S:(ec + 1) * S])
    nc.sync.dma_start(out=wv_t[:, :, :],
                        in_=wv.rearrange("p (c d) -> p c d", c=EC))
    nc.gpsimd.dma_start(out=wo_t[:, :, :],
                        in_=wo.rearrange("p (c e) -> p c e", c=2))
    nc.gpsimd.dma_start(out=bqk_t[:, 0, :], in_=bq[None, :])
    nc.gpsimd.dma_start(out=bqk_t[:, 1, :], in_=bk[None, :])
    nc.vector.memset(ones_t[:, :], 1.0)
    for kc in range(KC):
        nc.vector.memset(v_t[:, kc, :, D:D + 1], 1.0)


    def qk_part(dc, proj, sc, half, state={}):
        # psum[d, s] += W[e, d].T @ X^T[e, s]   (+ bias via K=1 matmul),
        # emitted in two halves so filler bursts stay small
        w_t, dst = ((wq_t, qT_t), (wk_t, kT_t))[proj]
        if half == 0:
            state[(dc, proj, sc)] = ps_big.tile(
                [128, 512], F32, tag="big", name="ps_qk")
        ps = state[(dc, proj, sc)]
        ecs = range(EC // 2) if half == 0 else range(EC // 2, EC)
        for ec in ecs:
            nc.tensor.matmul(
                ps[:, :],
                lhsT=w_t[:, ec, dc * 128:(dc + 1) * 128],
                rhs=xT_t[:, ec, sc * 512:(sc + 1) * 512],
                start=(ec == 0), stop=False)
        if half == 1:
            nc.tensor.matmul(
                ps[:, :],
                lhsT=bqk_t[:, proj, dc * 128:(dc + 1) * 128],
                rhs=ones_t[:, :],
                start=False, stop=True)
            nc.vector.tensor_copy(
                out=dst[:, dc, sc * 512:(sc + 1) * 512], in_=ps[:, :])
            del state[(dc, proj, sc)]

    def qk_group(dc, proj, sc):
        qk_part(dc, proj, sc, 0)
        qk_part(dc, proj, sc, 1)

    def v_proj():
        # psum[s, d] += X^T[e, s].T @ Wv[e, d]
        for kc in range(KC):
            ps = ps_acc.tile([128, 512], F32, tag="acc", bufs=4, name="ps_v")
            for ec in range(EC):
                nc.tensor.matmul(
                    ps[:, 0:DC],
                    lhsT=xT_t[:, ec, kc * 128:(kc + 1) * 128],
                    rhs=wv_t[:, ec, :],
                    start=(ec == 0), stop=(ec == EC - 1))
            nc.vector.tensor_copy(
                out=v_t[:, kc, :, 0:D],
                in_=ps[:, 0:DC].rearrange("p (h d) -> p h d", h=GH))

    def attention_scores(qc, hc, kcs=None):
        # Head pair (2*hc, 2*hc+1): head hp=0 on SBUF partitions 0-63, hp=1
        # on 64-127, so the two scores matmuls run as independent 64x128 PE
        # tiles and one ACTIVATE covers both heads' exp.
        pTs = []
        for kc in (kcs if kcs is not None else range(KC)):
            sco = ps_big.tile([128, 2, 512], F32, tag="big", name="sco")
            for hp in range(2):
                po = hp * 64
                nc.tensor.matmul(
                    sco[:, hp, :],
                    lhsT=kT_t[po:po + 64, hc, kc * 128:(kc + 1) * 128],
                    rhs=qT_t[po:po + 64, hc, qc * 512:(qc + 1) * 512],
                    start=True, stop=True)
            pT = sb_p.tile([128, 2, 512], MM_DT)
            nc.scalar.activation(
                out=pT[:, :, :], in_=sco[:, :, :], func=EXP_FUNC,
                scale=float(SCALE))
            pTs.append(pT)
        return pTs

    def pv_alloc():
        return [ps_acc.tile([128, 512], F32, tag="acc", bufs=4, name=f"acc{j}")
                for j in range(2)]

    def pv_kc(accs, hc, pTs, kc):
        for hp in range(2):
            h = 2 * hc + hp
            nc.tensor.matmul(
                accs[hp][0:D + 1, :],
                lhsT=v_t[:, kc, h, :],
                rhs=pTs[kc][:, hp, :],
                start=(kc == 0), stop=(kc == KC - 1))

    def attention_norm(qc, hc, accs):
        for hp in range(2):
            po = hp * 64
            rs = sb_norm.tile([1, 512], F32, tag="rs")
            nc.vector.tensor_copy(out=rs[:, :], in_=accs[hp][D:D + 1, :])
            inv_r = sb_norm.tile([1, 512], F32, tag="inv")
            nc.vector.reciprocal_approx_fast(out=inv_r[:, :], in_=rs[:, :])
            brd = sb_norm.tile([64, 512], F32, tag="brd")
            nc.gpsimd.partition_broadcast(brd[:, :], inv_r[:, :])
            nc.vector.tensor_mul(
                o_t[po:po + 64, hc, qc * 512:(qc + 1) * 512],
                accs[hp][0:D, :],
                brd[:, :])

    def attention_pv(qc, hc, pTs):
        accs = pv_alloc()
        for kc in range(KC):
            pv_kc(accs, hc, pTs, kc)
        attention_norm(qc, hc, accs)

    def y_group(qc, ec, tag="acc", bufs=4, copy_eng=None):
        # psum[e, s] += Wo[c, e].T @ O^T[c, s] for chunk (ec, qc)
        yp = ps_acc.tile([128, 512], F32, tag=tag, bufs=bufs, name="yp")
        for cc in range(2):
            nc.tensor.matmul(
                yp[:, :],
                lhsT=wo_t[:, cc, ec * 128:(ec + 1) * 128],
                rhs=o_t[:, cc, qc * 512:(qc + 1) * 512],
                start=(cc == 0), stop=(cc == 1))
        ys = sb_y.tile([128, 512], F32)
        if copy_eng == "scalar":
            nc.scalar.copy(out=ys[:, :], in_=yp[:, :])
        else:
            nc.vector.tensor_copy(out=ys[:, :], in_=yp[:, :])
        nc.sync.dma_start(
            out=yT[ec * 128:(ec + 1) * 128, qc * 512:(qc + 1) * 512],
            in_=ys[:, :])

    def y_proj(qc):
        for ec in range(EC):
            y_group(qc, ec)

    # Software-pipelined emission (= Tile priority order).  The exp stream
    # drives the schedule: each attention block emits scores+exp for (qc, hc)
    # at top priority while the PREVIOUS block's PV matmuls and filler work
    # (remaining projections, output-projection chunks) weave in at kc
    # granularity, so ScalarE never starves.
    blocks = [(0, 0), (1, 0), (2, 0), (3, 0), (0, 1), (1, 1), (2, 1), (3, 1)]

    def qk1(proj, sc, half):
        return lambda: qk_part(1, proj, sc, half)

    def qk0(proj, sc, half):
        return lambda: qk_part(0, proj, sc, half)

    # filler generators keyed by block index: list of (after_kc, fn)
    fillers = {
        1: [(3, qk0(0, 2, 0)), (5, qk0(0, 2, 1)),
            (11, qk0(0, 3, 0)), (13, qk0(0, 3, 1))],
        2: [(1, qk1(0, 0, 0)), (3, qk1(0, 0, 1)),
            (5, qk1(1, 0, 0)), (7, qk1(1, 0, 1)),
            (9, qk1(0, 1, 0)), (11, qk1(0, 1, 1)),
            (13, qk1(1, 1, 0)), (15, qk1(1, 1, 1))],
        3: [(1, qk1(0, 2, 0)), (3, qk1(0, 2, 1)),
            (5, qk1(1, 2, 0)), (7, qk1(1, 2, 1)),
            (9, qk1(0, 3, 0)), (11, qk1(0, 3, 1)),
            (13, qk1(1, 3, 0)), (15, qk1(1, 3, 1))],
        6: [(2 * i + 1, (lambda e: lambda: y_group(0, e))(i)) for i in range(EC)],
        7: [(2 * i + 1, (lambda e: lambda: y_group(1, e))(i)) for i in range(EC)],
    }

    # staged startup: emit first-block scores as soon as each kT s-chunk's
    # projection is emitted, so the exp stream starts ~20us earlier
    qk_group(0, 0, 0)
    qk_group(0, 1, 0)
    pts_prev = attention_scores(0, 0, range(0, 4))
    qk_group(0, 1, 1)
    pts_prev += attention_scores(0, 0, range(4, 8))
    qk_group(0, 1, 2)
    pts_prev += attention_scores(0, 0, range(8, 12))
    qk_group(0, 1, 3)
    pts_prev += attention_scores(0, 0, range(12, 16))
    qk_group(0, 0, 1)
    v_proj()
    prev_block = (0, 0)
    for bi in range(1, len(blocks)):
        qc, hc = blocks[bi]
        pqc, phc = prev_block
        accs = pv_alloc()
        pts_cur = []
        fl = dict((k, f) for k, f in fillers.get(bi, []))
        for kc in range(KC):
            pts_cur += attention_scores(qc, hc, [kc])
            pv_kc(accs, phc, pts_prev, kc)
            if kc in fl:
                fl[kc]()
        attention_norm(pqc, phc, accs)
        pts_prev = pts_cur
        prev_block = (qc, hc)
    # final block: PV + norm + remaining output projection
    accs = pv_alloc()
    for kc in range(KC):
        pv_kc(accs, prev_block[1], pts_prev, kc)
        if kc % 2 == 1:
            y_group(2, kc // 2)
    attention_norm(prev_block[0], prev_block[1], accs)
    for ec in range(EC):
        y_group(3, ec, copy_eng="scalar" if ec % 2 else None)

    if dbg is not None:
        for name, t in (("qT", qT_t), ("kT", kT_t), ("o", o_t)):
            f = sb_big.tile([128, 2, S], F32, name=f"dump_{name}")
            nc.vector.tensor_copy(out=f[:, :, :], in_=t[:, :, :])
            nc.sync.dma_start(out=dbg[name], in_=f.rearrange("p a b -> p (a b)"))
        fv = sb_big.tile([128, KC, GH, D + 1], F32, name="dump_v")
        nc.vector.tensor_copy(out=fv[:, :, :, :], in_=v_t[:, :, :, :])
        nc.sync.dma_start(out=dbg["v"], in_=fv.rearrange("p a b c -> p (a b c)"))


_cached_nc = None


def _build():
    nc = bacc.Bacc(trn_type="TRN2", target_bir_lowering=False)
    xT = nc.dram_tensor("xT", [128, EC * S], MM_DT, kind="ExternalInput").ap()
    wq = nc.dram_tensor("wq", [128, EC * DC], MM_DT, kind="ExternalInput").ap()
    wk = nc.dram_tensor("wk", [128, EC * DC], MM_DT, kind="ExternalInput").ap()
    wv = nc.dram_tensor("wv", [128, EC * DC], MM_DT, kind="ExternalInput").ap()
    wo = nc.dram_tensor("wo", [128, 2 * E], MM_DT, kind="ExternalInput").ap()
    bq = nc.dram_tensor("bq", [DC], MM_DT, kind="ExternalInput").ap()
    bk = nc.dram_tensor("bk", [DC], MM_DT, kind="ExternalInput").ap()
    yT = nc.dram_tensor("yT", [E, S], F32, kind="ExternalOutput").ap()
    dbg = None
    if DEBUG_DUMPS:
        dbg = {
            "qT": nc.dram_tensor("dbg_qT", [128, 2 * S], F32, kind="ExternalOutput").ap(),
            "kT": nc.dram_tensor("dbg_kT", [128, 2 * S], F32, kind="ExternalOutput").ap(),
            "o": nc.dram_tensor("dbg_o", [128, 2 * S], F32, kind="ExternalOutput").ap(),
            "v": nc.dram_tensor("dbg_v", [128, KC * GH * (D + 1)], F32, kind="ExternalOutput").ap(),
        }
    with tile.TileContext(nc) as tc:
        with ExitStack() as ctx:
            _emit(nc, tc, ctx, xT, wq, wk, wv, wo, bq, bk, yT, dbg)
    nc.compile()
    return nc


def get_nc():
    global _cached_nc
    if _cached_nc is None:
        _cached_nc = _build()
    return _cached_nc


def make_in_maps(inputs, wq, bq, wk, bk, wv, wo):
    in_maps = []
    for c in range(NCORES):
        b, g = divmod(c, GH)
        sl = slice(g * DC, (g + 1) * DC)
        def perm(a):
            # [C*128, N] -> [128, C*N] with SBUF chunk-major free dim
            cN = a.shape[0] // 128
            return np.ascontiguousarray(
                a.reshape(cN, 128, a.shape[1]).transpose(1, 0, 2).reshape(
                    128, cN * a.shape[1]))

        in_maps.append({
            "xT": round_f32r(perm(np.ascontiguousarray(inputs[b].T))),
            "wq": round_f32r(perm(wq[:, sl])),
            "wk": round_f32r(perm(wk[:, sl])),
            "wv": round_f32r(perm(wv[:, sl])),
            "wo": round_f32r(perm(wo[sl, :])),
            "bq": round_f32r(bq[sl]),
            "bk": round_f32r(bk[sl]),
        })
    return in_maps


def combine(results, wv_full, bv, wo_full, bo):
    y = np.zeros((B, S, E), np.float32)
    for c in range(NCORES):
        y[c // GH] += results[c]["yT"].T
    y += bv @ wo_full + bo
    return y


def kernel(inputs, wq, bq, wk, bk, wv, bv, wo, bo, _run_kwargs=None):
    inputs = np.asarray(inputs, np.float32)
    wq, bq = np.asarray(wq, np.float32), np.asarray(bq, np.float32)
    wk, bk = np.asarray(wk, np.float32), np.asarray(bk, np.float32)
    wv, bv = np.asarray(wv, np.float32), np.asarray(bv, np.float32)
    wo, bo = np.asarray(wo, np.float32), np.asarray(bo, np.float32)

    nc = get_nc()
    in_maps = make_in_maps(inputs, wq, bq, wk, bk, wv, wo)
    res = run_bass_kernel_spmd(nc, in_maps, list(range(NCORES)),
                               **(_run_kwargs or {}))
    y = combine(res.results, wv, bv, wo, bo)
    if _run_kwargs:
        kernel.last_result = res
    return y



# revision 11
# speedup vs baseline: 1.0363x; 1.0363x over previous
"""Multi-head self-attention (B=2, S=2048, E=1024, H=16, D=64) on 8 NeuronCores.

Sharding: core c -> (batch b = c // 4, head group g = c % 4).  Each core
computes Q/K/V projections for its 4 heads (column-parallel), attention, and
a partial output projection (row-parallel); the host sums the 4 partials per
batch.  All device activations live in "transposed space" (feature on the
partition dim) so every matmul contracts along partitions with no on-device
transposes:

  Q^T = Wq_g^T @ X^T          [256, 2048]  (e-chunk accumulated; bias via DVE)
  K^T = Wk_g^T @ X^T          [256, 2048]
  V   = X @ Wv_g              [2048, 256]  (natural; ones column appended)
  S^T = K_h @ Q_h^T / 8       [2048, 2048] per head (row-tiled 64x128 pairs)
  P^T = exp(S^T)              (softmax without max-subtraction: scores ~N(0,1))
  O'^T = [V_h | 1]^T @ P^T    [65, q]  (row 64 = softmax denominators)
  O^T  = O'[0:64] / O'[64]    (DVE reciprocal + GpSimd partition broadcast)
  Y^T  = Wo_g^T @ O^T         [1024, 2048] partial, host-summed per batch

The emission is a software pipeline paced by the ScalarE exp stream (the
critical path: 128 ACTIVATEs x ~1.15us = 147us).  Each slot (block, kc)
emits the scores matmul pair + exp at top priority; PV pairs and filler
granules (projection chunks, output-projection chunks) consume the PE slack
behind the exp stream, ordered by deadline.  Engines are pre-warmed (HAM
clock gate + ACT table load) during the input DMA window.

bv and bo are folded on the host (exact: softmax rows sum to 1, so
attn(V + bv) = attn(V) + bv, and the output projection is linear).
"""

from contextlib import ExitStack

import numpy as np

import concourse.bass as bass
import concourse.tile as tile
from concourse import bacc, mybir
from concourse.bass_utils import run_bass_kernel_spmd

B, S, E, H, D = 2, 2048, 1024, 16, 64
NCORES = 8
GH = 4            # heads per core
DC = GH * D       # head-dim columns per core (256)
EC = E // 128     # 8 e-chunks
KC = S // 128     # 16 k-chunks
F32 = mybir.dt.float32
MM_DT = mybir.dt.float16    # full-speed 16-bit matmul path (10-bit mantissa)
EXP_FUNC = mybir.ActivationFunctionType.Exp
SCALE = 1.0 / np.sqrt(np.float32(D))

BLOCKS = [(0, 0), (1, 0), (2, 0), (3, 0), (0, 1), (1, 1), (2, 1), (3, 1)]


def round_f32r(a):
    # Host-side conversion to the matmul dtype (RNE)
    if MM_DT == mybir.dt.float16:
        return np.ascontiguousarray(a, np.float32).astype(np.float16)
    if MM_DT == mybir.dt.bfloat16:
        import ml_dtypes
        return np.ascontiguousarray(a, np.float32).astype(ml_dtypes.bfloat16)
    return np.ascontiguousarray(a, np.float32)


DEBUG_DUMPS = False


def _emit(nc, tc, ctx, xT, wq, wk, wv, wo, bqk, yT, dbg=None):
    sb_big = ctx.enter_context(tc.tile_pool(name="sb_big", bufs=1))
    sb_p = ctx.enter_context(tc.tile_pool(name="sb_p", bufs=12))
    sb_norm = ctx.enter_context(tc.tile_pool(name="sb_norm", bufs=4))
    sb_y = ctx.enter_context(tc.tile_pool(name="sb_y", bufs=3))
    ps_sco = ctx.enter_context(tc.tile_pool(name="ps_sco", bufs=2, space="PSUM"))
    ps_acc = ctx.enter_context(tc.tile_pool(name="ps_acc", bufs=2, space="PSUM"))
    ps_fill = ctx.enter_context(tc.tile_pool(name="ps_fill", bufs=2, space="PSUM"))

    xT_t = sb_big.tile([128, EC, S], MM_DT)
    wq_t = sb_big.tile([128, EC, DC], MM_DT)
    wk_t = sb_big.tile([128, EC, DC], MM_DT)
    wv_t = sb_big.tile([128, EC, DC], MM_DT)
    wo_t = sb_big.tile([128, 2, E], MM_DT)
    bqk_t = sb_big.tile([128, 2, 2], F32)
    qT_t = sb_big.tile([128, 2, S], MM_DT)
    kT_t = sb_big.tile([128, 2, S], MM_DT)
    v_t = sb_big.tile([128, KC, GH, D + 1], MM_DT)
    o_t = sb_big.tile([128, 2, S], MM_DT)
    junk_a = sb_big.tile([1, 1], MM_DT)
    junk_b = sb_big.tile([1, 128], MM_DT)
    junk_o = sb_big.tile([1, 128], F32)

    # ---- engine warmup (runs during the input DMA window) ----------------
    # ScalarE: trigger the exp ACT table load (~2.7us) before the first real
    # ACTIVATE.  TensorE: ~4us of junk matmuls so the HAM clock-gate reaches
    # 8/8 (2.4 GHz) before the first projection matmul.
    nc.vector.memset(junk_a[:, :], 0.25)
    nc.vector.memset(junk_b[:, :], 0.25)
    nc.scalar.activation(out=junk_o[:, :], in_=junk_b[:, :], func=EXP_FUNC,
                         scale=1.0)
    wrm = ps_fill.tile([1, 128], F32, tag="fill", bufs=2, name="wrm")
    NWARM = 40
    for i in range(NWARM):
        nc.tensor.matmul(wrm[:, :], lhsT=junk_a[:, :], rhs=junk_b[:, :],
                         start=(i == 0), stop=(i == NWARM - 1))

    # ---- input DMA, deadline ordered, round-robin over 3 trigger queues --
    qs = [nc.sync, nc.gpsimd]
    qi = [0]

    def dma(out, in_):
        qs[qi[0] % 2].dma_start(out=out, in_=in_)
        qi[0] += 1

    wqr = wq.rearrange("p (c d) -> p c d", c=EC)
    wkr = wk.rearrange("p (c d) -> p c d", c=EC)
    # xT s-chunk sc=0 + first weight halves: gates the first scores+exp
    for ec in range(EC):
        dma(xT_t[:, ec, 0:512], xT[:, ec * S:ec * S + 512])
    dma(wq_t[:, :, 0:128], wqr[:, :, 0:128])
    dma(wk_t[:, :, 0:128], wkr[:, :, 0:128])
    dma(wv_t[:, :, :], wv.rearrange("p (c d) -> p c d", c=EC))
    dma(bqk_t[:, :, :], bqk.rearrange("p (a b) -> p a b", a=2))
    for sc in range(1, 4):
        for ec in range(EC):
            dma(xT_t[:, ec, sc * 512:(sc + 1) * 512],
                xT[:, ec * S + sc * 512:ec * S + (sc + 1) * 512])
    dma(wq_t[:, :, 128:256], wqr[:, :, 128:256])
    dma(wk_t[:, :, 128:256], wkr[:, :, 128:256])
    dma(wo_t[:, :, :], wo.rearrange("p (c e) -> p c e", c=2))
    nc.vector.memset(v_t[:, :, :, D:D + 1], 1.0)

    # ---- emission helpers ------------------------------------------------
    qk_state = {}

    def qk_half(dc, proj, sc, half):
        # psum[d, s] += W[e, d].T @ X^T[e, s], two 4-ec halves per part;
        # the PSUM->SBUF copy adds the bias (per-partition scalar) on DVE.
        w_t, dst = ((wq_t, qT_t), (wk_t, kT_t))[proj]
        if half == 0:
            qk_state[(dc, proj, sc)] = ps_fill.tile(
                [128, 512], F32, tag="fill", bufs=2, name="ps_qk")
        ps = qk_state[(dc, proj, sc)]
        for ec in (range(4) if half == 0 else range(4, 8)):
            nc.tensor.matmul(
                ps[:, :],
                lhsT=w_t[:, ec, dc * 128:(dc + 1) * 128],
                rhs=xT_t[:, ec, sc * 512:(sc + 1) * 512],
                start=(ec == 0), stop=(ec == EC - 1))
        if half == 1:
            nc.vector.tensor_scalar_add(
                dst[:, dc, sc * 512:(sc + 1) * 512], ps[:, :],
                bqk_t[:, proj, dc:dc + 1])
            del qk_state[(dc, proj, sc)]

    def v_piece(kc):
        # psum[s, d] += X^T[e, s].T @ Wv[e, d]
        ps = ps_fill.tile([128, 512], F32, tag="fill", bufs=2, name="ps_v")
        for ec in range(EC):
            nc.tensor.matmul(
                ps[:, 0:DC],
                lhsT=xT_t[:, ec, kc * 128:(kc + 1) * 128],
                rhs=wv_t[:, ec, :],
                start=(ec == 0), stop=(ec == EC - 1))
        nc.vector.tensor_copy(
            out=v_t[:, kc, :, 0:D],
            in_=ps[:, 0:DC].rearrange("p (h d) -> p h d", h=GH))

    def y_group(qc, ec):
        # psum[e, s] += Wo[c, e].T @ O^T[c, s]
        yp = ps_fill.tile([128, 512], F32, tag="fill", bufs=2, name="yp")
        for cc in range(2):
            nc.tensor.matmul(
                yp[:, :],
                lhsT=wo_t[:, cc, ec * 128:(ec + 1) * 128],
                rhs=o_t[:, cc, qc * 512:(qc + 1) * 512],
                start=(cc == 0), stop=(cc == 1))
        ys = sb_y.tile([128, 512], F32)
        nc.vector.tensor_copy(out=ys[:, :], in_=yp[:, :])
        nc.sync.dma_start(
            out=yT[ec * 128:(ec + 1) * 128, qc * 512:(qc + 1) * 512],
            in_=ys[:, :])

    pt_tiles = {}           # (bi, kc) -> (tile, idx) for PV consumption

    def scores_exp(bi, kc):
        # Head pair: hp=0 on K-partitions 0-63, hp=1 on 64-127 -> the two
        # 64x128 matmuls run as concurrent PE row-tiles; one ACTIVATE does
        # exp for both heads (N=1024).
        qc, hc = BLOCKS[bi]
        sco = ps_sco.tile([128, 2, 512], F32, tag="sco", bufs=2, name="sco")
        for hp in range(2):
            po = hp * 64
            nc.tensor.matmul(
                sco[:, hp, :],
                lhsT=kT_t[po:po + 64, hc, kc * 128:(kc + 1) * 128],
                rhs=qT_t[po:po + 64, hc, qc * 512:(qc + 1) * 512],
                start=True, stop=True)
        if kc % 2 == 0:
            pt_tiles[(bi, kc)] = (
                sb_p.tile([128, 2, 2, 512], MM_DT, name="pt"), 0)
        else:
            pt_tiles[(bi, kc)] = (pt_tiles[(bi, kc - 1)][0], 1)
        pt, idx = pt_tiles[(bi, kc)]
        nc.scalar.activation(
            out=pt[:, idx, :, :], in_=sco[:, :, :], func=EXP_FUNC,
            scale=float(SCALE))

    accs = {}               # bi -> [acc_hp0, acc_hp1]

    def pv_pair(bi, kc):
        qc, hc = BLOCKS[bi]
        if kc == 0:
            accs[bi] = [ps_acc.tile([128, 512], F32, tag="acc", bufs=2,
                                    name=f"acc{bi}_{hp}") for hp in range(2)]
        pt, idx = pt_tiles.pop((bi, kc))
        for hp in range(2):
            h = 2 * hc + hp
            nc.tensor.matmul(
                accs[bi][hp][0:D + 1, :],
                lhsT=v_t[:, kc, h, :],
                rhs=pt[:, idx, hp, :],
                start=(kc == 0), stop=(kc == KC - 1))

    def norm(bi):
        # Copy the PV accumulators out of PSUM immediately (frees the banks
        # for the next block's PV pair), then normalize from SBUF.
        qc, hc = BLOCKS[bi]
        a = accs.pop(bi)
        for hp in range(2):
            po = hp * 64
            raw = sb_norm.tile([64, 512], F32, tag=f"raw{hp}")
            nc.vector.tensor_copy(out=raw[:, :], in_=a[hp][0:D, :])
            rs = sb_norm.tile([1, 512], F32, tag="rs")
            nc.vector.tensor_copy(out=rs[:, :], in_=a[hp][D:D + 1, :])
            inv_r = sb_norm.tile([1, 512], F32, tag="inv")
            nc.vector.reciprocal_approx_fast(out=inv_r[:, :], in_=rs[:, :])
            brd = sb_norm.tile([64, 512], F32, tag="brd")
            nc.gpsimd.partition_broadcast(brd[:, :], inv_r[:, :])
            nc.vector.tensor_mul(
                o_t[po:po + 64, hc, qc * 512:(qc + 1) * 512],
                raw[:, :], brd[:, :])

    # ---- software pipeline ----------------------------------------------
    # Filler queue: (earliest global slot, closure), drained one granule per
    # slot.  Slots are numbered bi*16 + kc.  Deadlines (strict: Tile executes
    # per-engine in emission order, so a filler emitted too early stalls PE):
    #   K(0, sc): scores(0, kc) needs kT s-chunk kc//4 -> by slot 4*sc.
    #   V(kc): needed by PV(0, kc), consumed in block 1 at slot 16+kc.
    #   Q(0, qc): by block qc.  K(1, sc): by slot 64+4*sc.  Q(1, qc): by
    #   block 4+qc.  Y(qc): after norm(block 4+qc), queued dynamically.
    # DMA arrival also bounds emission from below (xT s-chunk sc lands at
    # ~3+2.9*sc us; wv at ~6us) - K(0,s3) is deliberately at slots 6-7.
    fillers = []

    def add(slot, fn, *args):
        fillers.append((slot, lambda: fn(*args)))

    add(0, qk_half, 0, 1, 1, 0)
    add(1, qk_half, 0, 1, 1, 1)           # K(0, s1) by slot 4
    add(2, qk_half, 0, 1, 2, 0)
    add(3, qk_half, 0, 1, 2, 1)           # K(0, s2) by slot 8
    add(4, v_piece, 0)
    add(5, v_piece, 1)
    add(6, qk_half, 0, 1, 3, 0)
    add(7, qk_half, 0, 1, 3, 1)           # K(0, s3) by slot 12
    add(8, v_piece, 2)
    add(9, v_piece, 3)
    add(10, v_piece, 4)
    add(11, qk_half, 0, 0, 1, 0)
    add(12, qk_half, 0, 0, 1, 1)          # Q(0, q1) by block 1
    add(13, v_piece, 5)
    for i in range(6, 16):                # V 6..15 JIT through block 1
        add(10 + i, v_piece, i)
    add(27, qk_half, 0, 0, 2, 0)
    add(28, qk_half, 0, 0, 2, 1)          # Q(0, q2) by block 2
    add(32, qk_half, 0, 0, 3, 0)
    add(33, qk_half, 0, 0, 3, 1)          # Q(0, q3) by block 3
    for sc in range(2):
        for h in range(2):
            add(34 + 2 * sc + h, qk_half, 1, 1, sc, h)   # K(1, s0-s1)
    for sc in range(2):
        for h in range(2):
            add(48 + 2 * sc + h, qk_half, 1, 1, 2 + sc, h)  # K(1, s2-s3)
    add(52, qk_half, 1, 0, 0, 0)
    add(53, qk_half, 1, 0, 0, 1)          # Q(1, q0) by block 4
    for qc in range(1, 4):                # Q(1, q1..3) by block 4+qc
        for h in range(2):
            add(16 * (3 + qc) + 2 * h, qk_half, 1, 0, qc, h)
    # Y(qc) granules are appended dynamically after norm(qc, hc=1).

    fillers.sort(key=lambda x: x[0])
    fq = list(fillers)

    # PV allowance per slot: block 0 none (V streaming in); block 1 drains
    # block 0's pairs at 1/slot; blocks 2-5 run at 1.5/slot to absorb the
    # one-block lag; steady lag ~1 slot from block 4 on.
    pv_allow = []
    for bi in range(8):
        for kc in range(KC):
            if bi == 0:
                pv_allow.append(0)
            elif 2 <= bi <= 5:
                pv_allow.append(2 if kc % 2 == 1 else 1)
            else:
                pv_allow.append(1)

    # startup: projections for the first block's scores
    for h in range(2):
        qk_half(0, 1, 0, h)       # K(0, s0)
    for h in range(2):
        qk_half(0, 0, 0, h)       # Q(0, q0)

    pv_seq = [(bi, kc) for bi in range(8) for kc in range(KC)]
    pv_head = 0
    pv_emitted = [0] * 8
    norms_done = set()

    for t in range(128):
        bi, kc = t // 16, t % 16
        scores_exp(bi, kc)
        allow = pv_allow[t]
        while allow > 0 and pv_head < len(pv_seq):
            pb, pk = pv_seq[pv_head]
            # only consume PV whose exp is already emitted (strictly past)
            if pb * 16 + pk >= t:
                break
            pv_pair(pb, pk)
            pv_emitted[pb] += 1
            pv_head += 1
            allow -= 1
            if pv_emitted[pb] == KC:
                norm(pb)
                norms_done.add(pb)
                if pb >= 4:       # hc=1 block done -> Y(qc) ready
                    qc = BLOCKS[pb][0]
                    for ec in range(EC):
                        fq.append((t + 1 + ec, lambda e=ec, q=qc: y_group(q, e)))
                    fq.sort(key=lambda x: x[0])
        # fillers: one granule per slot when due
        while fq and fq[0][0] <= t:
            _, fn = fq.pop(0)
            fn()
            break

    # tail: drain remaining PV, norms, fillers (Y chunks), then last Y
    while pv_head < len(pv_seq):
        pb, pk = pv_seq[pv_head]
        pv_pair(pb, pk)
        pv_emitted[pb] += 1
        pv_head += 1
        if pv_emitted[pb] == KC:
            norm(pb)
            norms_done.add(pb)
            if pb >= 4:
                qc = BLOCKS[pb][0]
                for ec in range(EC):
                    fq.append((0, lambda e=ec, q=qc: y_group(q, e)))
    for _, fn in fq:
        fn()

    if dbg is not None:
        for name, t in (("qT", qT_t), ("kT", kT_t), ("o", o_t)):
            nc.sync.dma_start(out=dbg[name], in_=t.rearrange("p a b -> p (a b)"))
        nc.sync.dma_start(out=dbg["v"], in_=v_t.rearrange("p a b c -> p (a b c)"))


_cached_nc = None


def _build():
    nc = bacc.Bacc(trn_type="TRN2", target_bir_lowering=False)
    xT = nc.dram_tensor("xT", [128, EC * S], MM_DT, kind="ExternalInput").ap()
    wq = nc.dram_tensor("wq", [128, EC * DC], MM_DT, kind="ExternalInput").ap()
    wk = nc.dram_tensor("wk", [128, EC * DC], MM_DT, kind="ExternalInput").ap()
    wv = nc.dram_tensor("wv", [128, EC * DC], MM_DT, kind="ExternalInput").ap()
    wo = nc.dram_tensor("wo", [128, 2 * E], MM_DT, kind="ExternalInput").ap()
    bqk = nc.dram_tensor("bqk", [128, 4], F32, kind="ExternalInput").ap()
    yT = nc.dram_tensor("yT", [E, S], F32, kind="ExternalOutput").ap()
    dbg = None
    if DEBUG_DUMPS:
        dbg = {
            "qT": nc.dram_tensor("dbg_qT", [128, 2 * S], MM_DT, kind="ExternalOutput").ap(),
            "kT": nc.dram_tensor("dbg_kT", [128, 2 * S], MM_DT, kind="ExternalOutput").ap(),
            "o": nc.dram_tensor("dbg_o", [128, 2 * S], MM_DT, kind="ExternalOutput").ap(),
            "v": nc.dram_tensor("dbg_v", [128, KC * GH * (D + 1)], MM_DT, kind="ExternalOutput").ap(),
        }
    with tile.TileContext(nc) as tc:
        with ExitStack() as ctx:
            _emit(nc, tc, ctx, xT, wq, wk, wv, wo, bqk, yT, dbg)
    nc.compile()
    return nc


def get_nc():
    global _cached_nc
    if _cached_nc is None:
        _cached_nc = _build()
    return _cached_nc


def make_in_maps(inputs, wq, bq, wk, bk, wv, wo):
    in_maps = []
    for c in range(NCORES):
        b, g = divmod(c, GH)
        sl = slice(g * DC, (g + 1) * DC)

        def perm(a):
            # [C*128, N] -> [128, C*N] with SBUF chunk-major free dim
            cN = a.shape[0] // 128
            return np.ascontiguousarray(
                a.reshape(cN, 128, a.shape[1]).transpose(1, 0, 2).reshape(
                    128, cN * a.shape[1]))

        bqk = np.stack([np.asarray(bq[sl], np.float32).reshape(2, 128).T,
                        np.asarray(bk[sl], np.float32).reshape(2, 128).T],
                       axis=1)          # [128, proj, dc]
        in_maps.append({
            "xT": round_f32r(perm(np.ascontiguousarray(inputs[b].T))),
            "wq": round_f32r(perm(wq[:, sl])),
            "wk": round_f32r(perm(wk[:, sl])),
            "wv": round_f32r(perm(wv[:, sl])),
            "wo": round_f32r(perm(wo[sl, :])),
            "bqk": np.ascontiguousarray(bqk.reshape(128, 4), np.float32),
        })
    return in_maps


def combine(results, wv_full, bv, wo_full, bo):
    y = np.zeros((B, S, E), np.float32)
    for c in range(NCORES):
        y[c // GH] += results[c]["yT"].T
    y += bv @ wo_full + bo
    return y


def kernel(inputs, wq, bq, wk, bk, wv, bv, wo, bo, _run_kwargs=None):
    inputs = np.asarray(inputs, np.float32)
    wq, bq = np.asarray(wq, np.float32), np.asarray(bq, np.float32)
    wk, bk = np.asarray(wk, np.float32), np.asarray(bk, np.float32)
    wv, bv = np.asarray(wv, np.float32), np.asarray(bv, np.float32)
    wo, bo = np.asarray(wo, np.float32), np.asarray(bo, np.float32)

    nc = get_nc()
    in_maps = make_in_maps(inputs, wq, bq, wk, bk, wv, wo)
    res = run_bass_kernel_spmd(nc, in_maps, list(range(NCORES)),
                               **(_run_kwargs or {}))
    y = combine(res.results, wv, bv, wo, bo)
    if _run_kwargs:
        kernel.last_result = res
    return y
